# revision 4
# baseline (speedup 1.0000x reference)
"""DecisionTransformer Trainium2 Bass kernel.

Strategy: data-parallel over batch (16 -> 8 cores x 2). Per core, activations
live feature-major (x^T: [128 part = channel%128, 8 d-tiles, 1536 tokens]) in
SBUF for the whole forward pass. All dense GEMMs run in float32r (tf32-like,
full PE rate at N>=256, ~1.6e-4 matmul error) with weights streamed from HBM.
Attention internals (Q/K/V head-scramble layout, probs) use bf16.

The reference's raw reshape (B,T,D)->(B,H,T,HD) scrambles tokens/channels;
with s = 16*t + c (c = channel block d//64), head h = s//768 and position
t' = s%768, q_head^T[hd, t'] = Q^T[64c+hd, t]. We materialize QS/KS/VS^T as
[64, 12288] "subtoken-major" tensors via 2 partition-half copies per PSUM
tile during QKV evacuation, which makes every attention operand a plain AP.
The inverse scramble for the output projection is absorbed into 16 strided
rhs matmuls (one per channel block c) accumulating in PSUM.
"""

import os
import threading

import numpy as np
import ml_dtypes

import concourse.bass as bass
from concourse import bacc
import concourse.mybir as mybir
import concourse.tile as tile
from concourse.bass_utils import run_bass_kernel_spmd

F32 = mybir.dt.float32
F32R = mybir.dt.float32r
BF16 = mybir.dt.bfloat16
AF = mybir.ActivationFunctionType
ALU = mybir.AluOpType

# Problem shape (hardcoded per contract)
B, SEQ, STATE, ACT_DIM, D, FFN, H, NBLK = 16, 256, 128, 18, 1024, 4096, 16, 6
T = 3 * SEQ            # 768 tokens per batch element
NC = 8                 # cores
BPC = B // NC          # 2 batch per core
TL = BPC * T           # 1536 tokens per core
DT = D // 128          # 8 d-tiles
S_B = T * (D // 64)    # 12288 subtokens per batch
QT = T // 128          # 6 q-tiles per head
MT1 = FFN // 128       # 32 FFN m-tiles
EPS = 1e-5

# bias pack columns
BQ, BK, BV, BO, B2C, N1G, N1B, N2G, N2B, B1C = 0, 8, 16, 24, 32, 40, 48, 56, 64, 72
# global const pack columns
GBS, GBR, GANG, GANB = 0, 8, 16, 24


def build(nblk=NBLK, tap=None):
    """Build the bass program. tap in (None, 'embed', 'block') adds xT dump."""
    nc = bacc.Bacc()

    # ---- DRAM io ----
    stateT = nc.dram_tensor("stateT", [128, 512], F32, kind="ExternalInput")
    rtgT = nc.dram_tensor("rtgT", [1, 512], F32, kind="ExternalInput")
    embAT = nc.dram_tensor("embAT", [1024, 512], F32, kind="ExternalInput")
    posT = nc.dram_tensor("posT", [1024, TL], F32, kind="ExternalInput")
    wsd = nc.dram_tensor("ws", [128, 1024], F32, kind="ExternalInput")
    wRd = nc.dram_tensor("wR", [1, 1024], F32, kind="ExternalInput")
    gconst = nc.dram_tensor("gconst", [128, 32], F32, kind="ExternalInput")
    woutd = nc.dram_tensor("wout", [128, DT, ACT_DIM], F32, kind="ExternalInput")
    maskd = nc.dram_tensor("maskc", [128, 128], F32, kind="ExternalInput")
    identd = nc.dram_tensor("identf", [128, 128], F32, kind="ExternalInput")
    identbd = nc.dram_tensor("identb", [128, 128], BF16, kind="ExternalInput")
    identhd = nc.dram_tensor("identh", [128, 64], BF16, kind="ExternalInput")
    onesd = nc.dram_tensor("onesc", [128, 2], F32, kind="ExternalInput")  # col0: ones
    ones1d = nc.dram_tensor("ones1", [1, 128], F32, kind="ExternalInput")
    wq_d, wk_d, wv_d, w1_d, w2_d, woc_d, bias_d = [], [], [], [], [], [], []
    for i in range(nblk):
        wq_d.append(nc.dram_tensor(f"wq{i}", [1024, 1024], F32, kind="ExternalInput"))
        wk_d.append(nc.dram_tensor(f"wk{i}", [1024, 1024], F32, kind="ExternalInput"))
        wv_d.append(nc.dram_tensor(f"wv{i}", [1024, 1024], F32, kind="ExternalInput"))
        w1_d.append(nc.dram_tensor(f"w1{i}", [1024, 4096], F32, kind="ExternalInput"))
        w2_d.append(nc.dram_tensor(f"w2{i}", [4096, 1024], BF16, kind="ExternalInput"))
        woc_d.append(nc.dram_tensor(f"woc{i}", [64, 16, 1024], BF16, kind="ExternalInput"))
        bias_d.append(nc.dram_tensor(f"bias{i}", [128, 104], F32, kind="ExternalInput"))
    logitsT = nc.dram_tensor("logitsT", [ACT_DIM, 512], F32, kind="ExternalOutput")
    xdump = None
    if tap is not None:
        xdump = nc.dram_tensor("xdump", [128, DT, TL], F32, kind="ExternalOutput")

    with tile.TileContext(nc) as tc:
        with (
            tc.tile_pool(name="konst", bufs=1) as kp,
            tc.tile_pool(name="big", bufs=1) as bp,
            tc.tile_pool(name="dbuf", bufs=2) as dp,
            tc.tile_pool(name="tbuf", bufs=3) as tp,
            tc.tile_pool(name="ps2", bufs=2, space="PSUM") as ps2,
            tc.tile_pool(name="ps1", bufs=1, space="PSUM") as ps1,
        ):
            # ---- persistent tiles ----
            x = bp.tile([128, DT, TL], F32R, tag="x")
            tileA = bp.tile([128, S_B], BF16, tag="tileA")  # rows 0:64 QS^T, 64:128 VS^T
            tileB = bp.tile([128, S_B], BF16, tag="tileB")  # rows 0:64 KS^T, 64:128 O^T
            vs = bp.tile([128, 96, 64], BF16, tag="vs")     # VS: [s-tile, 128 s, 64 hd]
            hbuf = bp.tile([128, MT1, 512], BF16, tag="h")

            mask = kp.tile([128, 128], F32, tag="mask")
            nc.sync.dma_start(mask[:], maskd[:])
            identb = kp.tile([128, 128], BF16, tag="identb")
            nc.sync.dma_start(identb[:], identbd[:])
            identh = kp.tile([128, 64], BF16, tag="identh")
            nc.sync.dma_start(identh[:], identhd[:])
            gc = kp.tile([128, 32], F32, tag="gconst")
            nc.sync.dma_start(gc[:], gconst[:])
            ones128 = kp.tile([128, 1], F32R, tag="ones128")
            nc.sync.dma_start(ones128[:], onesd[:, 0:1].bitcast(F32R))
            ones1 = kp.tile([1, 128], F32R, tag="ones1")
            nc.sync.dma_start(ones1[:], ones1d[:].bitcast(F32R))
            wout_sb = kp.tile([128, DT, ACT_DIM], F32R, tag="wout")
            nc.sync.dma_start(wout_sb[:], woutd[:].bitcast(F32R))
            epst = kp.tile([1, 1], F32, tag="eps")
            nc.vector.memset(epst[:], EPS)

            # ---- embedding ----
            ws_sb = kp.tile([128, 1024], F32R, tag="ws")
            nc.sync.dma_start(ws_sb[:], wsd[:].bitcast(F32R))
            wR_sb = kp.tile([1, 1024], F32R, tag="wR")
            nc.sync.dma_start(wR_sb[:], wRd[:].bitcast(F32R))
            st_sb = kp.tile([128, 512], F32R, tag="stateT")
            nc.sync.dma_start(st_sb[:], stateT[:].bitcast(F32R))
            rtg_sb = kp.tile([1, 512], F32R, tag="rtgT")
            nc.sync.dma_start(rtg_sb[:], rtgT[:].bitcast(F32R))

            embA3 = embAT.rearrange("(ko ki) j -> ki ko j", ki=128)
            pos3 = posT.rearrange("(ko ki) t -> ki ko t", ki=128)
            for dt in range(DT):
                # action embeds: staged load then strided copy into x[:, dt, 2::3]
                stg = tp.tile([128, 512], F32, tag="bc512")
                nc.sync.dma_start(stg[:], embA3[:, dt, :])
                nc.vector.tensor_copy(x[:, dt, 2::3], stg[:])
                # state embeds -> x[:, dt, 1::3]
                p = ps2.tile([128, 512], F32, tag="mm512")
                nc.tensor.matmul(p[:], ws_sb[:, dt * 128:(dt + 1) * 128], st_sb[:],
                                 start=True, stop=True)
                nc.scalar.activation(x[:, dt, 1::3], p[:], AF.Tanh,
                                     bias=gc[:, GBS + dt:GBS + dt + 1], scale=1.0 / 255.0)
                # rtg embeds -> x[:, dt, 0::3]
                p = ps2.tile([128, 512], F32, tag="mm512")
                nc.tensor.matmul(p[:], wR_sb[:, dt * 128:(dt + 1) * 128], rtg_sb[:],
                                 start=True, stop=True)
                nc.scalar.activation(x[:, dt, 0::3], p[:], AF.Tanh,
                                     bias=gc[:, GBR + dt:GBR + dt + 1], scale=1.0)
                # add positional embeddings (chunks of 512)
                for ch in range(3):
                    pt = tp.tile([128, 512], F32, tag="bc512")
                    nc.sync.dma_start(pt[:], pos3[:, dt, ch * 512:(ch + 1) * 512])
                    nc.vector.tensor_tensor(x[:, dt, ch * 512:(ch + 1) * 512],
                                            x[:, dt, ch * 512:(ch + 1) * 512], pt[:],
                                            ALU.add)

            if tap == "embed":
                nc.sync.dma_start(xdump[:], x[:].bitcast(F32))

            # ---- layernorm routine (in-place on x) ----
            def emit_ln(bias_tile, gcol, bcol):
                for ch in range(3):
                    t0 = ch * 512
                    psum_s = ps2.tile([128, 512], F32, tag="mm512")
                    psum_q = ps2.tile([128, 512], F32, tag="mm512")
                    for kt in range(DT):
                        nc.tensor.matmul(psum_s[0:1, :], ones128[:], x[:, kt, t0:t0 + 512],
                                         start=(kt == 0), stop=(kt == DT - 1))
                    for kt in range(DT):
                        sq = dp.tile([128, 512], F32R, tag="sqtmp")
                        nc.vector.tensor_tensor(sq[:], x[:, kt, t0:t0 + 512],
                                                x[:, kt, t0:t0 + 512], ALU.mult)
                        nc.tensor.matmul(psum_q[0:1, :], ones128[:], sq[:],
                                         start=(kt == 0), stop=(kt == DT - 1))
                    mrow = kp.tile([1, 512], F32R, tag="mrow")
                    nc.vector.tensor_scalar_mul(mrow[:], psum_s[0:1, :], 1.0 / D)
                    msq = kp.tile([1, 512], F32, tag="msq")
                    nc.vector.tensor_tensor(msq[:], mrow[:].bitcast(F32),
                                            mrow[:].bitcast(F32), ALU.mult)
                    rrow = kp.tile([1, 512], F32R, tag="rrow")
                    nc.vector.scalar_tensor_tensor(rrow[:], psum_q[0:1, :], 1.0 / D,
                                                   msq[:], ALU.mult, ALU.subtract)
                    # r = rsqrt(var+eps) = exp(-0.5*ln(var+eps)); ln&exp share a table set
                    nc.scalar.activation(rrow[:], rrow[:].bitcast(F32), AF.Ln,
                                         bias=epst[:])
                    nc.scalar.activation(rrow[:], rrow[:].bitcast(F32), AF.Exp, scale=-0.5)
                    # broadcast m, r across partitions via K=1 matmul
                    bcm = tp.tile([128, 512], F32, tag="bc512")
                    bcr = tp.tile([128, 512], F32, tag="bc512")
                    pb = ps2.tile([128, 512], F32, tag="mm512")
                    nc.tensor.matmul(pb[:], ones1[:], mrow[:], start=True, stop=True)
                    nc.scalar.copy(bcm[:], pb[:])
                    pb = ps2.tile([128, 512], F32, tag="mm512")
                    nc.tensor.matmul(pb[:], ones1[:], rrow[:], start=True, stop=True)
                    nc.scalar.copy(bcr[:], pb[:])
                    for dt in range(DT):
                        xs = x[:, dt, t0:t0 + 512]
                        nc.vector.tensor_tensor(xs, xs, bcm[:], ALU.subtract)
                        nc.vector.tensor_tensor(xs, xs, bcr[:], ALU.mult)
                        nc.vector.tensor_scalar(xs, xs,
                                                bias_tile[:, gcol + dt:gcol + dt + 1],
                                                bias_tile[:, bcol + dt:bcol + dt + 1],
                                                ALU.mult, ALU.add)

            # ---- transformer blocks ----
            for blk in range(nblk):
                bias = dp.tile([128, 104], F32, tag="bias")
                nc.sync.dma_start(bias[:], bias_d[blk][:])
                w3 = [wq_d[blk].rearrange("(ko ki) m -> ki ko m", ki=128),
                      wk_d[blk].rearrange("(ko ki) m -> ki ko m", ki=128),
                      wv_d[blk].rearrange("(ko ki) m -> ki ko m", ki=128)]
                w13 = w1_d[blk].rearrange("(ko ki) m -> ki ko m", ki=128)
                w23 = w2_d[blk].rearrange("(ko ki) m -> ki ko m", ki=128)

                for b in range(BPC):
                    tb0 = b * T
                    # --- QKV projections, evacuated into scrambled layouts ---
                    for ti in range(3):
                        bcol = (BQ, BK, BV)[ti]
                        if ti == 0:
                            drows, dbase = tileA, 0
                        elif ti == 1:
                            drows, dbase = tileB, 0
                        else:
                            drows, dbase = tileA, 64
                        for dt in range(DT):
                            wt = dp.tile([128, DT, 128], F32R, tag="wk8")
                            nc.sync.dma_start(
                                wt[:], w3[ti][:, :, dt * 128:(dt + 1) * 128].bitcast(F32R))
                            ps = ps2.tile([128, T], F32, tag="mm768")
                            for c0, cw in ((0, 512), (512, 256)):
                                for kt in range(DT):
                                    nc.tensor.matmul(
                                        ps[:, c0:c0 + cw], wt[:, kt, :],
                                        x[:, kt, tb0 + c0:tb0 + c0 + cw],
                                        start=(kt == 0), stop=(kt == DT - 1))
                            for half in range(2):
                                c = 2 * dt + half
                                dest = drows[dbase:dbase + 64, c::16]
                                nc.scalar.activation(
                                    dest, ps[64 * half:64 * half + 64, :], AF.Identity,
                                    bias=bias[64 * half:64 * half + 64, bcol + dt:bcol + dt + 1])

                    # --- build VS (token-major subtokens) by transposing VS^T ---
                    for sg in range(12):
                        pt = ps2.tile([128, 512], BF16, tag="mm512")
                        for j in range(8):
                            stile = sg * 8 + j
                            nc.tensor.transpose(
                                pt[:, j * 64:(j + 1) * 64],
                                tileA[64:128, stile * 128:(stile + 1) * 128],
                                identh[64:128, :])
                        nc.vector.tensor_copy(vs[:, sg * 8:sg * 8 + 8, :], pt[:])

                    # --- attention per head ---
                    for h in range(H):
                        hb = h * T
                        den = dp.tile([128, QT], F32, tag="den")
                        rec = dp.tile([128, QT], F32, tag="rec")
                        oT = ps1.tile([64, T], F32, tag="pso")
                        for qt in range(QT):
                            L = (qt + 1) * 128
                            ps = ps2.tile([128, T], F32, tag="mm768")
                            for c0 in range(0, L, 512):
                                cw = min(512, L - c0)
                                nc.tensor.matmul(
                                    ps[:, c0:c0 + cw],
                                    tileA[0:64, hb + qt * 128:hb + (qt + 1) * 128],
                                    tileB[0:64, hb + c0:hb + c0 + cw],
                                    start=True, stop=True)
                            nc.vector.tensor_tensor(ps[:, qt * 128:L], ps[:, qt * 128:L],
                                                    mask[:], ALU.add)
                            a = dp.tile([128, T], BF16, tag="abuf")
                            nc.scalar.activation(a[:, :L], ps[:, :L], AF.Exp,
                                                 accum_out=den[:, qt:qt + 1])
                            nc.vector.reciprocal(rec[:, qt:qt + 1], den[:, qt:qt + 1])
                            nc.vector.tensor_scalar_mul(a[:, :L], a[:, :L], rec[:, qt:qt + 1])
                            atb = dp.tile([128, QT, 128], BF16, tag="atb")
                            for g0 in range(0, qt + 1, 4):
                                gn = min(4, qt + 1 - g0)
                                pt = ps2.tile([128, 512], BF16, tag="mm512")
                                for j in range(gn):
                                    nc.tensor.transpose(
                                        pt[:, j * 128:(j + 1) * 128],
                                        a[:, (g0 + j) * 128:(g0 + j + 1) * 128],
                                        identb[:])
                                nc.vector.tensor_copy(atb[:, g0:g0 + gn, :],
                                                      pt[:, :gn * 128])
                            for kc in range(qt + 1):
                                nc.tensor.matmul(
                                    oT[:, qt * 128:(qt + 1) * 128],
                                    vs[:, h * 6 + kc, :], atb[:, kc, :],
                                    start=(kc == 0), stop=(kc == qt))
                        nc.vector.tensor_copy(tileB[64:128, hb:hb + T], oT[:])

                    # --- output projection (+bo, +residual) ---
                    for dt in range(DT):
                        wocs = dp.tile([128, 16, 128], BF16, tag="woc")
                        nc.sync.dma_start(wocs[64:128, :, :],
                                          woc_d[blk][:, :, dt * 128:(dt + 1) * 128])
                        for c0, cw in ((0, 512), (512, 256)):
                            po = ps2.tile([128, 512], F32, tag="mm512")
                            for c in range(16):
                                nc.tensor.matmul(
                                    po[:, :cw], wocs[64:128, c, :],
                                    tileB[64:128, 16 * c0 + c:16 * (c0 + cw - 1) + c + 1:16],
                                    start=(c == 0), stop=(c == 15))
                            xs = x[:, dt, tb0 + c0:tb0 + c0 + cw]
                            nc.vector.scalar_tensor_tensor(
                                xs, po[:, :cw], bias[:, BO + dt:BO + dt + 1], xs,
                                ALU.add, ALU.add)

                # --- LN1 ---
                emit_ln(bias, N1G, N1B)

                # --- FFN ---
                for ch in range(3):
                    t0 = ch * 512
                    for mt in range(MT1):
                        w1t = dp.tile([128, DT, 128], F32R, tag="wk8")
                        nc.sync.dma_start(w1t[:],
                                          w13[:, :, mt * 128:(mt + 1) * 128].bitcast(F32R))
                        pf = ps2.tile([128, 512], F32, tag="mm512")
                        for kt in range(DT):
                            nc.tensor.matmul(pf[:], w1t[:, kt, :], x[:, kt, t0:t0 + 512],
                                             start=(kt == 0), stop=(kt == DT - 1))
                        nc.scalar.activation(hbuf[:, mt, :], pf[:], AF.Relu,
                                             bias=bias[:, B1C + mt:B1C + mt + 1])
                    for dt in range(DT):
                        pf2 = ps2.tile([128, 512], F32, tag="mm512")
                        for khalf in range(2):
                            w2t = dp.tile([128, 16, 128], BF16, tag="w2s")
                            nc.sync.dma_start(
                                w2t[:], w23[:, khalf * 16:khalf * 16 + 16,
                                            dt * 128:(dt + 1) * 128])
                            for kj in range(16):
                                kt = khalf * 16 + kj
                                nc.tensor.matmul(pf2[:], w2t[:, kj, :], hbuf[:, kt, :],
                                                 start=(kt == 0), stop=(kt == 31))
                        xs = x[:, dt, t0:t0 + 512]
                        nc.vector.scalar_tensor_tensor(
                            xs, pf2[:], bias[:, B2C + dt:B2C + dt + 1], xs,
                            ALU.add, ALU.add)

                # --- LN2 ---
                emit_ln(bias, N2G, N2B)

                if tap == "block" and blk == nblk - 1:
                    nc.sync.dma_start(xdump[:], x[:].bitcast(F32))

            # ---- final layernorm + unembed ----
            emit_ln(gc, GANG, GANB)
            pu = ps2.tile([128, 512], F32, tag="mm512")
            for kt in range(DT):
                nc.tensor.matmul(pu[0:ACT_DIM, :], wout_sb[:, kt, :], x[:, kt, 1::3],
                                 start=(kt == 0), stop=(kt == DT - 1))
            lsb = kp.tile([ACT_DIM, 512], F32, tag="lout")
            nc.vector.tensor_copy(lsb[:], pu[0:ACT_DIM, :])
            nc.sync.dma_start(logitsT[:], lsb[:])

    nc.finalize()
    return nc


# ------------------------ host side ------------------------

_cache = {}
_lock = threading.Lock()


def _get_nc(nblk=NBLK, tap=None):
    key = (nblk, tap)
    with _lock:
        if key not in _cache:
            _cache[key] = build(nblk, tap)
        return _cache[key]


def _f32(a):
    return np.ascontiguousarray(np.asarray(a, dtype=np.float32))


def _prep_inputs(state, action, rtg, timestep, params, nblk=NBLK):
    state = _f32(state); rtg = _f32(rtg)
    action = np.asarray(action); timestep = np.asarray(timestep)
    P = params
    tanh_emb_a = np.tanh(_f32(P['emb_a']))                      # [18, 1024]
    pos_emb = _f32(P['pos_emb'])[0, :T, :]                       # [768, 1024]
    gpe = _f32(P['global_pos_emb'])[0]                           # [4096, 1024]

    def colpack(v, n):
        return np.ascontiguousarray(_f32(v).reshape(n, 128).T)   # [128, n]

    shared = {
        'ws': _f32(P['ws']),
        'wR': _f32(P['wR']),
        'gconst': np.ascontiguousarray(np.concatenate([
            colpack(P['bs'], 8), colpack(P['bR'], 8),
            colpack(P['an_g'], 8), colpack(P['an_b'], 8)], axis=1)),
        'wout': np.ascontiguousarray(
            _f32(P['w_out']).reshape(DT, 128, ACT_DIM).transpose(1, 0, 2)),
        'maskc': np.where(np.tril(np.ones((128, 128), bool)), 0.0,
                          -1e30).astype(np.float32),
        'identf': np.eye(128, dtype=np.float32),
        'identb': np.eye(128).astype(ml_dtypes.bfloat16),
        'identh': np.concatenate([np.zeros((64, 64)), np.eye(64)],
                                 axis=0).astype(ml_dtypes.bfloat16),
        'onesc': np.ones((128, 2), dtype=np.float32),
        'ones1': np.ones((1, 128), dtype=np.float32),
    }
    for i in range(nblk):
        blk = P['blocks'][i]
        shared[f'wq{i}'] = _f32(blk['wq'])
        shared[f'wk{i}'] = _f32(blk['wk'])
        shared[f'wv{i}'] = _f32(blk['wv'])
        shared[f'w1{i}'] = _f32(blk['w1'])
        shared[f'w2{i}'] = _f32(blk['w2']).astype(ml_dtypes.bfloat16)
        shared[f'woc{i}'] = np.ascontiguousarray(
            _f32(blk['wo']).reshape(16, 64, 1024).transpose(1, 0, 2)
        ).astype(ml_dtypes.bfloat16)
        shared[f'bias{i}'] = np.ascontiguousarray(np.concatenate([
            colpack(blk['bq'], 8), colpack(blk['bk'], 8), colpack(blk['bv'], 8),
            colpack(blk['bo'], 8), colpack(blk['b2'], 8),
            colpack(blk['n1g'], 8), colpack(blk['n1b'], 8),
            colpack(blk['n2g'], 8), colpack(blk['n2b'], 8),
            colpack(blk['b1'], 32)], axis=1))

    in_maps = []
    for c in range(NC):
        bsl = slice(BPC * c, BPC * (c + 1))
        st = state[bsl].reshape(BPC * SEQ, STATE)                # [512, 128]
        rt = rtg[bsl].reshape(BPC * SEQ, 1)
        act = action[bsl].reshape(BPC * SEQ).astype(np.int64)
        embA = tanh_emb_a[act]                                   # [512, 1024]
        pos_list = []
        for j in range(BPC):
            ts_idx = int(timestep[BPC * c + j, 0, 0])
            pos_list.append(gpe[ts_idx][None, :] + pos_emb)      # [768, 1024]
        pos = np.concatenate(pos_list, axis=0)                   # [1536, 1024]
        m = dict(shared)
        m['stateT'] = np.ascontiguousarray(st.T)
        m['rtgT'] = np.ascontiguousarray(rt.T)
        m['embAT'] = np.ascontiguousarray(embA.T)
        m['posT'] = np.ascontiguousarray(pos.T)
        in_maps.append(m)
    return in_maps


def run(state, action, rtg, timestep, params, nblk=NBLK, tap=None, trace=False):
    nc = _get_nc(nblk, tap)
    in_maps = _prep_inputs(state, action, rtg, timestep, params, nblk)
    res = run_bass_kernel_spmd(nc, in_maps, core_ids=list(range(NC)), trace=trace)
    outs = []
    for c in range(NC):
        lt = res.results[c]['logitsT']                           # [18, 512]
        outs.append(lt.T.reshape(BPC, SEQ, ACT_DIM))
    logits = np.concatenate(outs, axis=0).astype(np.float32)
    extra = [res.results[c].get('xdump') for c in range(NC)] if tap else None
    return logits, extra, res


def kernel(state, action, rtg, timestep, params):
    logits, _, _ = run(state, action, rtg, timestep, params,
                       trace=bool(os.environ.get('BASS_TRACE')))
    return logits
